# revision 9
# baseline (speedup 1.0000x reference)
"""Trainium2 Bass kernel for nn_LinearGML2.

Computes out[b, k] = || (x_b - w_k) @ L_k ||_2 for K=256 per-class
lower-triangular matrices L_k (diag = L_diags**2, strict lower = L_lower),
B=1024, d=512.  Sharded over classes: 32 per core.

Math: ||(x - w_k) L_k||^2 = ||x L_k||^2 - 2 x . g_k + c_k with
g_k = L_k L_k^T w_k^T, c_k = ||w_k L_k||^2 (host-precomputed vectors).
The big matmul's only per-class operand is L; x is class-independent, so
there is no per-class VectorE prep pass.

All matmul inputs are fp8e4 with DoubleRow perf mode (contract 256 rows
per instruction at 0.5 cyc per output column): the 512x512 triangular L
splits into 2 d-superblocks x 4 e-blocks with 6 nonzero pairs.

The sum-of-squares epilogue is the wall (every psum element must cross
one engine at ~1 elem/cyc/partition, and TRN2 ops can read at most ONE
psum operand), so classes split across two balanced pipelines:
  * KA "ACT-path" classes, transposed layout u^T[e, b]: ScalarE squares
    psum -> SBUF fp8; TensorE reduces the squares over e via one-hot-lhsT
    DoubleRow matmuls that accumulate ssq directly into a persistent
    [32, 512] psum row per class, on top of the -2 x.g cross terms
    (G' = -2G packed like a 33rd class).  c_k enters via the final Sqrt
    activation's per-partition bias.  Output leaves k-major.
  * ND "DVE-path" classes, normal layout u[b, e]: VectorE bn_stats per
    (class, chunk) + batched decode; cross terms ride the same shared-x
    lhsT into a one-bank psum region; c_k via a host-broadcast tile.
    Output leaves b-major.
Two b-half passes for the ACT path keep PSUM within 8 banks:
4 (transposed pipeline) + 1 (acc) + 2 (DVE pipeline) + 1 (cross).
"""

from contextlib import ExitStack

import numpy as np

import concourse.bass as bass  # noqa: F401  (import keeps bass registered)
import concourse.tile as tile
from concourse import bacc, mybir
from concourse._compat import with_exitstack
from concourse.alu_op_type import AluOpType
from concourse.bass_utils import run_bass_kernel_spmd

K_CLASSES = 256
D = 512
B = 1024
N_CORES = 8
KC = K_CLASSES // N_CORES  # classes per core = 32
P = 128
BH = B // 2  # b-half = 512
KA = 18  # ACT-path classes (transposed + one-hot reduce)
ND = KC - KA  # DVE-path classes (normal + bn_stats)
NCH = B // P  # chunks = 8

_FP8 = mybir.dt.float8e4
_F32 = mybir.dt.float32
_SQUARE = mybir.ActivationFunctionType.Square
_SQRT = mybir.ActivationFunctionType.Sqrt
_DR = mybir.MatmulPerfMode.DoubleRow

FP8_NP = mybir.dt.np(_FP8)

# transposed mains order (j = d-superblock, c = e-block); j=1 (rows
# 256:512) feeds every e-block, j=0 only e-blocks 0,1.  c=2,3 first so
# psum pair-tile 1 completes early and its drain overlaps the j=0 work.
MAIN_ORDER = ((1, 2), (1, 3), (1, 0), (1, 1), (0, 0), (0, 1))


@with_exitstack
def _gml2_kernel(ctx: ExitStack, tc: "tile.TileContext", out1, out2, xq, lt0q, lt1q, gq, oh, cb, cb2):
    nc = tc.nc
    const = ctx.enter_context(tc.tile_pool(name="const", bufs=1))
    ltpool = ctx.enter_context(tc.tile_pool(name="lt", bufs=1))
    sqpool = ctx.enter_context(tc.tile_pool(name="sq", bufs=6))
    dcpool = ctx.enter_context(tc.tile_pool(name="dc", bufs=2))
    mmT = ctx.enter_context(tc.tile_pool(name="mmT", bufs=2, space="PSUM"))
    upp = ctx.enter_context(tc.tile_pool(name="up", bufs=2, space="PSUM"))
    accp = ctx.enter_context(tc.tile_pool(name="acc", bufs=1, space="PSUM"))
    crp = ctx.enter_context(tc.tile_pool(name="cr", bufs=1, space="PSUM"))

    xq_sb = const.tile([P, 2, 2, B], _FP8, name="xq_sb")
    gq_sb = const.tile([P, 2, 2, KC], _FP8, name="gq_sb")
    oh_sb = const.tile([P, 2, KA, KC], _FP8, name="oh_sb")
    cb_sb = const.tile([KA, 1], _F32, name="cb_sb")
    cb2_sb = const.tile([P, NCH, ND], _F32, name="cb2_sb")
    bnst = const.tile([P, NCH, ND, 6], _F32, name="bnst")
    sqd = const.tile([P, NCH, ND], _F32, name="sqd")
    osb1 = const.tile([KA, 2, BH], _F32, name="osb1")
    osb2 = const.tile([P, NCH, ND], _F32, name="osb2")

    for j in (1, 0):
        for s in (0, 1):
            nc.gpsimd.dma_start(xq_sb[:, j, s, :], xq[j, s])
    nc.gpsimd.dma_start(gq_sb[:, :, :, :], gq.rearrange("j s p m -> p j s m"))
    nc.gpsimd.dma_start(oh_sb[:, :, :, :], oh)
    nc.gpsimd.dma_start(cb_sb[:, :], cb)
    nc.gpsimd.dma_start(cb2_sb[:, :, :], cb2)

    lt0 = [None] * KC
    lt1 = [None] * KC
    # DMA order follows consumption order: interleave ACT/DVE class needs.
    def _load_lt(k, q):
        lt0[k] = ltpool.tile([P, 2, 256], _FP8, tag=f"lt0_{k}", name=f"lt0_{k}")
        lt1[k] = ltpool.tile([P, 2, D], _FP8, tag=f"lt1_{k}", name=f"lt1_{k}")
        q.dma_start(lt1[k][:, :, :], lt1q[k])
        q.dma_start(lt0[k][:, :, :], lt0q[k])

    order = []
    dve_seq = list(range(KA, KC))
    ai, di = 0, 0
    for i in range(KC):
        if i % 5 == 2 and di < ND:
            order.append(dve_seq[di]); di += 1
        elif ai < KA:
            order.append(ai); ai += 1
        elif di < ND:
            order.append(dve_seq[di]); di += 1
    for i, k in enumerate(order):
        _load_lt(k, nc.sync if i % 2 == 0 else nc.scalar)

    creg = crp.tile([P, NCH, ND], _F32, name="creg")

    # ---- ACT-path (transposed) emitters --------------------------------
    def t_mains(k, h):
        pt = [
            mmT.tile([P, 2, BH], _F32, tag="mmT", name=f"mmT{k}_{h}{t}")
            for t in (0, 1)
        ]
        for j, c in MAIN_ORDER:
            lsrc = lt1[k] if j == 1 else lt0[k]
            nc.tensor.matmul(
                pt[c // 2][:, c % 2, :],
                lsrc[:, :, c * 128 : (c + 1) * 128],
                xq_sb[:, j, :, h * BH : (h + 1) * BH],
                start=(j == 1),
                stop=(j == 0) if c <= 1 else (j == 1),
                perf_mode=_DR,
            )
        return pt

    def t_drain(k, h, pt):
        st = []
        for t in (0, 1):
            s = sqpool.tile([P, 2, BH], _FP8, tag="sq", name=f"s{k}_{h}{t}")
            nc.scalar.activation(s[:, :, :], pt[t][:, :, :], _SQUARE)
            st.append(s)
        return st

    def t_reduce(k, st, acc_t, stop):
        ohk = oh_sb[:, :, k, :]
        for t in (0, 1):
            nc.tensor.matmul(
                acc_t[:, :],
                ohk,
                st[t],
                start=False,
                stop=(stop and t == 1),
                perf_mode=_DR,
                skip_group_check=True,
            )

    # ---- DVE-path (normal orientation) emitter -------------------------
    def u_class(k, kfirst):
        m = k - KA
        for ch in range(NCH):
            up = upp.tile([P, D], _F32, tag="up", name=f"up{k}_{ch}")
            for j in (1, 0):
                lhsT = xq_sb[:, j, :, ch * P : (ch + 1) * P]
                nc.tensor.matmul(
                    up[:, :] if j == 1 else up[:, 0:256],
                    lhsT,
                    lt1[k][:, :, :] if j == 1 else lt0[k][:, :, :],
                    start=(j == 1),
                    stop=(j == 0),
                    perf_mode=_DR,
                )
                if kfirst:
                    nc.tensor.matmul(
                        creg[:, ch, :],
                        lhsT,
                        gq_sb[:, j, :, KA:KC],
                        start=(ch == 0 and j == 1),
                        stop=(ch == NCH - 1 and j == 0),
                        perf_mode=_DR,
                        skip_group_check=True,
                    )
            nc.vector.bn_stats(bnst[:, ch, m, :], up[:, :])

    def decode(ks):
        # sum(u^2) = M2_e + M2_o + 256 * (mean_e^2 + mean_o^2)
        me, m2e = bnst[:, :, ks, 1], bnst[:, :, ks, 2]
        mo, m2o = bnst[:, :, ks, 4], bnst[:, :, ks, 5]
        nk = len(range(*ks.indices(ND)))
        t1f = dcpool.tile([P, NCH, ND], _F32, tag="t1", name="t1")
        t2f = dcpool.tile([P, NCH, ND], _F32, tag="t2", name="t2")
        t1 = t1f[:, :, 0:nk]
        t2 = t2f[:, :, 0:nk]
        nc.vector.tensor_mul(t1, me, me)
        nc.vector.tensor_mul(t2, mo, mo)
        nc.vector.tensor_add(t1, t1, t2)
        nc.vector.tensor_add(t2, m2e, m2o)
        nc.vector.scalar_tensor_tensor(
            sqd[:, :, ks], t1, float(D // 2), t2, AluOpType.mult, AluOpType.add
        )

    # ---- schedule ------------------------------------------------------
    # per b-half pass: cross matmuls open the acc group, then the KA
    # ACT-classes with 7 DVE-classes woven between drain(k) and
    # reduce(k-1) so the PE always has main work while ScalarE drains.
    dve_pass = (dve_seq[: ND // 2], dve_seq[ND // 2 :])
    weave = {1, 4, 6, 9, 11, 14, 16}
    for h in (0, 1):
        acc_t = accp.tile([KC, BH], _F32, tag="acc", name=f"acc{h}")
        for j in (0, 1):
            nc.tensor.matmul(
                acc_t[:, :],
                gq_sb[:, j, :, :],
                xq_sb[:, j, :, h * BH : (h + 1) * BH],
                start=(j == 0),
                stop=False,
                perf_mode=_DR,
                skip_group_check=True,
            )
        dq = list(dve_pass[h])
        prev = None
        for i in range(KA):
            pt = t_mains(i, h)
            st = t_drain(i, h, pt)
            if i in weave and dq:
                k = dq.pop(0)
                u_class(k, kfirst=(h == 0 and k == KA))
            if prev is not None:
                t_reduce(i - 1, prev, acc_t, stop=False)
            prev = st
        while dq:
            u_class(dq.pop(0), kfirst=False)
        t_reduce(KA - 1, prev, acc_t, stop=True)
        nc.scalar.activation(osb1[:, h, :], acc_t[0:KA, :], _SQRT, bias=cb_sb[:, :])
        decode(slice(h * (ND // 2), (h + 1) * (ND // 2) if h == 0 else ND))

    # ssq_dve = sqd - 2*cross + c, then sqrt
    t1f = dcpool.tile([P, NCH, ND], _F32, tag="t1", name="t1f")
    # creg already holds -2 * x.g (host packs G' = -2G)
    nc.vector.scalar_tensor_tensor(
        t1f[:, :, :], creg[:, :, :], 1.0, cb2_sb[:, :, :],
        AluOpType.mult, AluOpType.add,
    )
    nc.vector.tensor_add(t1f[:, :, :], t1f[:, :, :], sqd[:, :, :])
    nc.scalar.activation(osb2[:, :, :], t1f[:, :, :], _SQRT)

    nc.sync.dma_start(out1, osb1[:, :, :])
    nc.sync.dma_start(out2, osb2[:, :, :])


_CACHE: dict = {}


def build_nc():
    if "nc" in _CACHE:
        return _CACHE["nc"]
    nc = bacc.Bacc("TRN2", target_bir_lowering=False, debug=False, num_devices=N_CORES)
    xq = nc.dram_tensor("xq", [2, 2, P, B], _FP8, kind="ExternalInput").ap()
    lt0q = nc.dram_tensor("lt0q", [KC, P, 2, 256], _FP8, kind="ExternalInput").ap()
    lt1q = nc.dram_tensor("lt1q", [KC, P, 2, D], _FP8, kind="ExternalInput").ap()
    gq = nc.dram_tensor("gq", [2, 2, P, KC], _FP8, kind="ExternalInput").ap()
    oh = nc.dram_tensor("oh", [P, 2, KA, KC], _FP8, kind="ExternalInput").ap()
    cb = nc.dram_tensor("cb", [KA, 1], _F32, kind="ExternalInput").ap()
    cb2 = nc.dram_tensor("cb2", [P, NCH, ND], _F32, kind="ExternalInput").ap()
    out1 = nc.dram_tensor("out1", [KA, 2, BH], _F32, kind="ExternalOutput").ap()
    out2 = nc.dram_tensor("out2", [P, NCH, ND], _F32, kind="ExternalOutput").ap()
    with tile.TileContext(nc) as tc:
        _gml2_kernel(tc, out1, out2, xq, lt0q, lt1q, gq, oh, cb, cb2)
    nc.compile()
    _CACHE["nc"] = nc
    return nc


def host_prep(inputs, weight, L_diags, L_lower):
    """Layout/dtype transforms + per-class g/c correction vectors."""
    x = np.asarray(inputs, dtype=np.float32)
    w = np.asarray(weight, dtype=np.float64).reshape(K_CLASSES, D)
    ld = np.asarray(L_diags, dtype=np.float64)
    ll = np.asarray(L_lower, dtype=np.float64)

    lmat = np.zeros((K_CLASSES, D, D), dtype=np.float64)
    ri, ci = np.tril_indices(D, k=-1)
    lmat[:, ri, ci] = ll
    dd = np.arange(D)
    lmat[:, dd, dd] = ld * ld

    # v_k = w_k L_k ; g_k = L_k v_k ; c_k = ||v_k||^2
    v = np.einsum("kd,kde->ke", w, lmat)
    g = np.einsum("kde,ke->kd", lmat, v)
    c = np.einsum("ke,ke->k", v, v)

    lmat32 = lmat.astype(np.float32)
    # weights element (p, s, m) = L[d = 256j + 128s + p, e]
    lt0q = np.ascontiguousarray(
        lmat32[:, 0:256, 0:256].reshape(K_CLASSES, 2, P, 256).transpose(0, 2, 1, 3)
    ).astype(FP8_NP)
    lt1q = np.ascontiguousarray(
        lmat32[:, 256:512, :].reshape(K_CLASSES, 2, P, D).transpose(0, 2, 1, 3)
    ).astype(FP8_NP)

    xq = np.ascontiguousarray(x.T.reshape(2, 2, P, B)).astype(FP8_NP)
    gT = (-2.0 * g).T.astype(np.float32).reshape(2, 2, P, K_CLASSES)  # (j, s, p, k)
    # one-hot column k among KC=32 (padded so the k-slice stride is 16-aligned)
    oh = np.broadcast_to(
        np.eye(KA, KC, dtype=np.float32)[None, None], (P, 2, KA, KC)
    )
    oh = np.ascontiguousarray(oh).astype(FP8_NP)
    return xq, lt0q, lt1q, gT, oh, c.astype(np.float32)


def make_in_maps(xq, lt0q, lt1q, gT, oh, c):
    in_maps = []
    for core in range(N_CORES):
        sl = slice(core * KC, (core + 1) * KC)
        cc = c[sl]
        cb2 = np.broadcast_to(cc[KA:][None, None, :], (P, NCH, ND))
        in_maps.append(
            {
                "xq": xq,
                "lt0q": np.ascontiguousarray(lt0q[sl]),
                "lt1q": np.ascontiguousarray(lt1q[sl]),
                "gq": np.ascontiguousarray(gT[:, :, :, sl]).astype(FP8_NP),
                "oh": oh,
                "cb": np.ascontiguousarray(cc[:KA].reshape(KA, 1)),
                "cb2": np.ascontiguousarray(cb2),
            }
        )
    return in_maps


def kernel(inputs, weight, L_diags, L_lower, **run_kwargs):
    packed = host_prep(inputs, weight, L_diags, L_lower)
    nc = build_nc()
    in_maps = make_in_maps(*packed)
    res = run_bass_kernel_spmd(nc, in_maps, core_ids=list(range(N_CORES)), **run_kwargs)
    out = np.empty((B, K_CLASSES), dtype=np.float32)
    for core in range(N_CORES):
        k0 = core * KC
        blk1 = np.asarray(res.results[core]["out1"]).astype(np.float32).reshape(KA, B)
        out[:, k0 : k0 + KA] = blk1.T
        blk2 = np.asarray(res.results[core]["out2"]).astype(np.float32)  # [P, NCH, ND]
        out[:, k0 + KA : k0 + KC] = blk2.transpose(1, 0, 2).reshape(B, ND)
    if run_kwargs:
        _CACHE["last_result"] = res
    return out


# revision 10
# speedup vs baseline: 1.1411x; 1.1411x over previous
"""Trainium2 Bass kernel for nn_LinearGML2.

Computes out[b, k] = || (x_b - w_k) @ L_k ||_2 for K=256 per-class
lower-triangular matrices L_k (diag = L_diags**2, strict lower = L_lower),
B=1024, d=512.  Sharded over classes: 32 per core.

Math: ||(x - w_k) L_k||^2 = ||x L_k||^2 - 2 x . g_k + c_k with
g_k = L_k L_k^T w_k^T, c_k = ||w_k L_k||^2 (host-precomputed vectors).
The big matmul's only per-class operand is L; x is class-independent, so
there is no per-class VectorE prep pass.

All matmul inputs are fp8e4 with DoubleRow perf mode (contract 256 rows
per instruction at 0.5 cyc per output column): the 512x512 triangular L
splits into 2 d-superblocks x 4 e-blocks with 6 nonzero pairs.

The sum-of-squares epilogue is the wall (every psum element must cross
one engine at ~1 elem/cyc/partition, and TRN2 ops can read at most ONE
psum operand), so classes split across two balanced pipelines:
  * KA "ACT-path" classes, transposed layout u^T[e, b]: ScalarE squares
    psum -> SBUF fp8; TensorE reduces the squares over e via one-hot-lhsT
    DoubleRow matmuls that accumulate ssq directly into a persistent
    [32, 512] psum row per class, on top of the -2 x.g cross terms
    (G' = -2G packed like a 33rd class).  c_k enters via the final Sqrt
    activation's per-partition bias.  Output leaves k-major.
  * ND "DVE-path" classes, normal layout u[b, e]: VectorE bn_stats per
    (class, chunk) + batched decode; cross terms ride the same shared-x
    lhsT into a one-bank psum region; c_k via a host-broadcast tile.
    Output leaves b-major.
Two b-half passes for the ACT path keep PSUM within 8 banks:
4 (transposed pipeline) + 1 (acc) + 2 (DVE pipeline) + 1 (cross).
"""

from contextlib import ExitStack

import numpy as np

import concourse.bass as bass  # noqa: F401  (import keeps bass registered)
import concourse.tile as tile
from concourse import bacc, mybir
from concourse._compat import with_exitstack
from concourse.alu_op_type import AluOpType
from concourse.bass_utils import run_bass_kernel_spmd

K_CLASSES = 256
D = 512
B = 1024
N_CORES = 8
KC = K_CLASSES // N_CORES  # classes per core = 32
P = 128
BH = B // 2  # b-half = 512
KA = 18  # ACT-path classes (transposed + one-hot reduce)
ND = KC - KA  # DVE-path classes (normal + bn_stats)
NCH = B // P  # chunks = 8

_FP8 = mybir.dt.float8e4
_F32 = mybir.dt.float32
_SQUARE = mybir.ActivationFunctionType.Square
_SQRT = mybir.ActivationFunctionType.Sqrt
_DR = mybir.MatmulPerfMode.DoubleRow

FP8_NP = mybir.dt.np(_FP8)

# transposed mains order (j = d-superblock, c = e-block); j=1 (rows
# 256:512) feeds every e-block, j=0 only e-blocks 0,1.  c=2,3 first so
# psum pair-tile 1 completes early and its drain overlaps the j=0 work.
MAIN_ORDER = ((1, 2), (1, 3), (1, 0), (1, 1), (0, 0), (0, 1))


@with_exitstack
def _gml2_kernel(ctx: ExitStack, tc: "tile.TileContext", out1, out2, xq, lt0q, lt1q, gq, oh, cb, cb2):
    nc = tc.nc
    const = ctx.enter_context(tc.tile_pool(name="const", bufs=1))
    ltpool = ctx.enter_context(tc.tile_pool(name="lt", bufs=1))
    sqpool = ctx.enter_context(tc.tile_pool(name="sq", bufs=6))
    dcpool = ctx.enter_context(tc.tile_pool(name="dc", bufs=2))
    mmT = ctx.enter_context(tc.tile_pool(name="mmT", bufs=2, space="PSUM"))
    upp = ctx.enter_context(tc.tile_pool(name="up", bufs=2, space="PSUM"))
    accp = ctx.enter_context(tc.tile_pool(name="acc", bufs=1, space="PSUM"))
    crp = ctx.enter_context(tc.tile_pool(name="cr", bufs=1, space="PSUM"))

    xq_sb = const.tile([P, 2, 2, B], _FP8, name="xq_sb")
    gq_sb = const.tile([P, 2, 2, KC], _FP8, name="gq_sb")
    oh_sb = const.tile([P, 2, KA, KC], _FP8, name="oh_sb")
    cb_sb = const.tile([KA, 1], _F32, name="cb_sb")
    cb2_sb = const.tile([P, NCH, ND], _F32, name="cb2_sb")
    bnst = const.tile([P, NCH, ND, 6], _F32, name="bnst")
    sqd = const.tile([P, NCH, ND], _F32, name="sqd")
    osb1 = const.tile([KA, 2, BH], _F32, name="osb1")
    osb2 = const.tile([P, NCH, ND], _F32, name="osb2")

    for j in (1, 0):
        for s in (0, 1):
            nc.gpsimd.dma_start(xq_sb[:, j, s, :], xq[j, s])
    nc.gpsimd.dma_start(gq_sb[:, :, :, :], gq.rearrange("j s p m -> p j s m"))
    nc.gpsimd.dma_start(oh_sb[:, :, :, :], oh)
    nc.gpsimd.dma_start(cb_sb[:, :], cb)
    nc.gpsimd.dma_start(cb2_sb[:, :, :], cb2)

    lt0 = [None] * KC
    lt1 = [None] * KC
    # DMA order follows consumption order: interleave ACT/DVE class needs.
    def _load_lt(k, q):
        lt0[k] = ltpool.tile([P, 2, 256], _FP8, tag=f"lt0_{k}", name=f"lt0_{k}")
        lt1[k] = ltpool.tile([P, 2, D], _FP8, tag=f"lt1_{k}", name=f"lt1_{k}")
        q.dma_start(lt1[k][:, :, :], lt1q[k])
        q.dma_start(lt0[k][:, :, :], lt0q[k])

    order = []
    dve_seq = list(range(KA, KC))
    ai, di = 0, 0
    for i in range(KC):
        if i % 5 == 2 and di < ND:
            order.append(dve_seq[di]); di += 1
        elif ai < KA:
            order.append(ai); ai += 1
        elif di < ND:
            order.append(dve_seq[di]); di += 1
    # lt DMAs stay off the Scalar queue: ScalarE is the drain engine.
    for i, k in enumerate(order):
        _load_lt(k, nc.sync if i % 2 == 0 else nc.gpsimd)

    creg = crp.tile([P, NCH, ND], _F32, name="creg")

    # ---- ACT-path (transposed) emitters --------------------------------
    def t_mains(k, h):
        pt = [
            mmT.tile([P, 2, BH], _F32, tag="mmT", name=f"mmT{k}_{h}{t}")
            for t in (0, 1)
        ]
        for j, c in MAIN_ORDER:
            lsrc = lt1[k] if j == 1 else lt0[k]
            nc.tensor.matmul(
                pt[c // 2][:, c % 2, :],
                lsrc[:, :, c * 128 : (c + 1) * 128],
                xq_sb[:, j, :, h * BH : (h + 1) * BH],
                start=(j == 1),
                stop=(j == 0) if c <= 1 else (j == 1),
                perf_mode=_DR,
            )
        return pt

    def t_drain(k, h, pt):
        st = []
        for t in (0, 1):
            s = sqpool.tile([P, 2, BH], _FP8, tag="sq", name=f"s{k}_{h}{t}")
            nc.scalar.activation(s[:, :, :], pt[t][:, :, :], _SQUARE)
            st.append(s)
        return st

    def t_reduce(k, st, acc_t, stop):
        ohk = oh_sb[:, :, k, :]
        for t in (0, 1):
            nc.tensor.matmul(
                acc_t[:, :],
                ohk,
                st[t],
                start=False,
                stop=(stop and t == 1),
                perf_mode=_DR,
                skip_group_check=True,
            )

    # ---- DVE-path (normal orientation) emitter -------------------------
    def u_class(k, kfirst):
        m = k - KA
        for ch in range(NCH):
            up = upp.tile([P, D], _F32, tag="up", name=f"up{k}_{ch}")
            for j in (1, 0):
                lhsT = xq_sb[:, j, :, ch * P : (ch + 1) * P]
                nc.tensor.matmul(
                    up[:, :] if j == 1 else up[:, 0:256],
                    lhsT,
                    lt1[k][:, :, :] if j == 1 else lt0[k][:, :, :],
                    start=(j == 1),
                    stop=(j == 0),
                    perf_mode=_DR,
                )
                if kfirst:
                    nc.tensor.matmul(
                        creg[:, ch, :],
                        lhsT,
                        gq_sb[:, j, :, KA:KC],
                        start=(ch == 0 and j == 1),
                        stop=(ch == NCH - 1 and j == 0),
                        perf_mode=_DR,
                        skip_group_check=True,
                    )
            nc.vector.bn_stats(bnst[:, ch, m, :], up[:, :])

    def decode(ks):
        # sum(u^2) = M2_e + M2_o + 256 * (mean_e^2 + mean_o^2)
        me, m2e = bnst[:, :, ks, 1], bnst[:, :, ks, 2]
        mo, m2o = bnst[:, :, ks, 4], bnst[:, :, ks, 5]
        nk = len(range(*ks.indices(ND)))
        t1f = dcpool.tile([P, NCH, ND], _F32, tag="t1", name="t1")
        t2f = dcpool.tile([P, NCH, ND], _F32, tag="t2", name="t2")
        t1 = t1f[:, :, 0:nk]
        t2 = t2f[:, :, 0:nk]
        nc.vector.tensor_mul(t1, me, me)
        nc.vector.tensor_mul(t2, mo, mo)
        nc.vector.tensor_add(t1, t1, t2)
        nc.vector.tensor_add(t2, m2e, m2o)
        nc.vector.scalar_tensor_tensor(
            sqd[:, :, ks], t1, float(D // 2), t2, AluOpType.mult, AluOpType.add
        )

    # ---- schedule ------------------------------------------------------
    # per b-half pass: cross matmuls open the acc group, then the KA
    # ACT-classes with 7 DVE-classes woven between drain(k) and
    # reduce(k-1) so the PE always has main work while ScalarE drains.
    dve_pass = (dve_seq[: ND // 2], dve_seq[ND // 2 :])
    weave = {1, 4, 6, 9, 11, 14, 16}
    for h in (0, 1):
        acc_t = accp.tile([KC, BH], _F32, tag="acc", name=f"acc{h}")
        for j in (0, 1):
            nc.tensor.matmul(
                acc_t[:, :],
                gq_sb[:, j, :, :],
                xq_sb[:, j, :, h * BH : (h + 1) * BH],
                start=(j == 0),
                stop=False,
                perf_mode=_DR,
                skip_group_check=True,
            )
        dq = list(dve_pass[h])
        prev = None
        for i in range(KA):
            pt = t_mains(i, h)
            st = t_drain(i, h, pt)
            if i in weave and dq:
                k = dq.pop(0)
                u_class(k, kfirst=(h == 0 and k == KA))
            if prev is not None:
                t_reduce(i - 1, prev, acc_t, stop=False)
            prev = st
        while dq:
            u_class(dq.pop(0), kfirst=False)
        t_reduce(KA - 1, prev, acc_t, stop=True)
        nc.scalar.activation(osb1[:, h, :], acc_t[0:KA, :], _SQRT, bias=cb_sb[:, :])
        decode(slice(h * (ND // 2), (h + 1) * (ND // 2) if h == 0 else ND))

    # ssq_dve = sqd - 2*cross + c, then sqrt
    t1f = dcpool.tile([P, NCH, ND], _F32, tag="t1", name="t1f")
    # creg already holds -2 * x.g (host packs G' = -2G)
    nc.vector.scalar_tensor_tensor(
        t1f[:, :, :], creg[:, :, :], 1.0, cb2_sb[:, :, :],
        AluOpType.mult, AluOpType.add,
    )
    nc.vector.tensor_add(t1f[:, :, :], t1f[:, :, :], sqd[:, :, :])
    nc.scalar.activation(osb2[:, :, :], t1f[:, :, :], _SQRT)

    nc.sync.dma_start(out1, osb1[:, :, :])
    nc.sync.dma_start(out2, osb2[:, :, :])


_CACHE: dict = {}


def build_nc():
    if "nc" in _CACHE:
        return _CACHE["nc"]
    nc = bacc.Bacc("TRN2", target_bir_lowering=False, debug=False, num_devices=N_CORES)
    xq = nc.dram_tensor("xq", [2, 2, P, B], _FP8, kind="ExternalInput").ap()
    lt0q = nc.dram_tensor("lt0q", [KC, P, 2, 256], _FP8, kind="ExternalInput").ap()
    lt1q = nc.dram_tensor("lt1q", [KC, P, 2, D], _FP8, kind="ExternalInput").ap()
    gq = nc.dram_tensor("gq", [2, 2, P, KC], _FP8, kind="ExternalInput").ap()
    oh = nc.dram_tensor("oh", [P, 2, KA, KC], _FP8, kind="ExternalInput").ap()
    cb = nc.dram_tensor("cb", [KA, 1], _F32, kind="ExternalInput").ap()
    cb2 = nc.dram_tensor("cb2", [P, NCH, ND], _F32, kind="ExternalInput").ap()
    out1 = nc.dram_tensor("out1", [KA, 2, BH], _F32, kind="ExternalOutput").ap()
    out2 = nc.dram_tensor("out2", [P, NCH, ND], _F32, kind="ExternalOutput").ap()
    with tile.TileContext(nc) as tc:
        _gml2_kernel(tc, out1, out2, xq, lt0q, lt1q, gq, oh, cb, cb2)
    nc.compile()
    _CACHE["nc"] = nc
    return nc


def host_prep(inputs, weight, L_diags, L_lower):
    """Layout/dtype transforms + per-class g/c correction vectors."""
    x = np.asarray(inputs, dtype=np.float32)
    w = np.asarray(weight, dtype=np.float64).reshape(K_CLASSES, D)
    ld = np.asarray(L_diags, dtype=np.float64)
    ll = np.asarray(L_lower, dtype=np.float64)

    lmat = np.zeros((K_CLASSES, D, D), dtype=np.float64)
    ri, ci = np.tril_indices(D, k=-1)
    lmat[:, ri, ci] = ll
    dd = np.arange(D)
    lmat[:, dd, dd] = ld * ld

    # v_k = w_k L_k ; g_k = L_k v_k ; c_k = ||v_k||^2
    v = np.einsum("kd,kde->ke", w, lmat)
    g = np.einsum("kde,ke->kd", lmat, v)
    c = np.einsum("ke,ke->k", v, v)

    lmat32 = lmat.astype(np.float32)
    # weights element (p, s, m) = L[d = 256j + 128s + p, e]
    lt0q = np.ascontiguousarray(
        lmat32[:, 0:256, 0:256].reshape(K_CLASSES, 2, P, 256).transpose(0, 2, 1, 3)
    ).astype(FP8_NP)
    lt1q = np.ascontiguousarray(
        lmat32[:, 256:512, :].reshape(K_CLASSES, 2, P, D).transpose(0, 2, 1, 3)
    ).astype(FP8_NP)

    xq = np.ascontiguousarray(x.T.reshape(2, 2, P, B)).astype(FP8_NP)
    gT = (-2.0 * g).T.astype(np.float32).reshape(2, 2, P, K_CLASSES)  # (j, s, p, k)
    # one-hot column k among KC=32 (padded so the k-slice stride is 16-aligned)
    oh = np.broadcast_to(
        np.eye(KA, KC, dtype=np.float32)[None, None], (P, 2, KA, KC)
    )
    oh = np.ascontiguousarray(oh).astype(FP8_NP)
    return xq, lt0q, lt1q, gT, oh, c.astype(np.float32)


def make_in_maps(xq, lt0q, lt1q, gT, oh, c):
    in_maps = []
    for core in range(N_CORES):
        sl = slice(core * KC, (core + 1) * KC)
        cc = c[sl]
        cb2 = np.broadcast_to(cc[KA:][None, None, :], (P, NCH, ND))
        in_maps.append(
            {
                "xq": xq,
                "lt0q": np.ascontiguousarray(lt0q[sl]),
                "lt1q": np.ascontiguousarray(lt1q[sl]),
                "gq": np.ascontiguousarray(gT[:, :, :, sl]).astype(FP8_NP),
                "oh": oh,
                "cb": np.ascontiguousarray(cc[:KA].reshape(KA, 1)),
                "cb2": np.ascontiguousarray(cb2),
            }
        )
    return in_maps


def kernel(inputs, weight, L_diags, L_lower, **run_kwargs):
    packed = host_prep(inputs, weight, L_diags, L_lower)
    nc = build_nc()
    in_maps = make_in_maps(*packed)
    res = run_bass_kernel_spmd(nc, in_maps, core_ids=list(range(N_CORES)), **run_kwargs)
    out = np.empty((B, K_CLASSES), dtype=np.float32)
    for core in range(N_CORES):
        k0 = core * KC
        blk1 = np.asarray(res.results[core]["out1"]).astype(np.float32).reshape(KA, B)
        out[:, k0 : k0 + KA] = blk1.T
        blk2 = np.asarray(res.results[core]["out2"]).astype(np.float32)  # [P, NCH, ND]
        out[:, k0 + KA : k0 + KC] = blk2.transpose(1, 0, 2).reshape(B, ND)
    if run_kwargs:
        _CACHE["last_result"] = res
    return out
